# revision 1
# baseline (speedup 1.0000x reference)
# GCN aggregation (10 layers of normalized scatter-add SpMV) on 8 NeuronCores.
#
# Formulation: with u = D^{-1/2} h, each layer is u' = D^{-1}(A^T u + u) and
# c5_l = ||h_l||^2 = sum(deg * u_l^2).
#
# Sharding: destinations dealt round-robin (per in-degree class) to the 8
# cores (baseline's newid layout). Gather of u[src] is done with the Ant
# custom dma_gather at 256B granularity: each edge slot fetches the 64-value
# line of the u table containing its source, on 4 SWDGE queues in parallel.
# A one-hot mask (built on-the-fly with is_equal against an iota row) selects
# the right element inside each line, fused with the per-run segment sum via
# tensor_tensor_reduce. Self-loop contribution is an extra slot per dest.
# One NEFF runs all 10 layers; u round-trips through AllGather buffers.
import numpy as np

N = 100000
E = 6400000
NDEV = 8
P = 128
L_LAYERS = 10
RUN = P * NDEV
W = (N + RUN - 1) // RUN          # 98 runs -> dest cols per partition
WU = NDEV * W                     # 784 full-table cols per partition
TBL0 = NDEV * P * W               # 100352 padded node slots
NLINES = TBL0 // 64               # 1568 lines of 64 fp32 (256B)
MAXLP = 64                        # max slots per gather piece
NQ = 4                            # SWDGE queues

_cache = {}


def legalize_waits(nc):
    # walrus here rejects >1 sem wait per instruction: split extras onto
    # single-wait NoOp carriers inserted before, on the same engine.
    import bass_rust
    import concourse.mybir as mybir
    n = 0
    for blk in nc.m.functions[0].blocks:
        insts = blk.instructions
        i = 0
        while i < len(insts):
            inst = insts[i]
            si = inst.sync_info
            if si is not None and len(si.on_wait) > 1:
                waits = list(si.on_wait)
                si.on_wait = [waits[-1]]
                pre = []
                for w in waits[:-1]:
                    n += 1
                    nop = mybir.InstNoOp(name=f"I-waitfix-{n}", ins=[], outs=[],
                                         text_hint="waitfix")
                    nop.engine = inst.engine
                    nop.sync_info = bass_rust.SyncInfo(on_wait=[w], on_update=[])
                    pre.append(nop)
                insts[i:i] = pre
                i += len(pre)
            i += 1
    return n


def _host_prep(h, edge_index):
    row = np.asarray(edge_index[0], dtype=np.int64)
    col = np.asarray(edge_index[1], dtype=np.int64)
    deg_nl = np.bincount(col, minlength=N).astype(np.int64)
    deg = deg_nl + 1

    order = np.argsort(-deg_nl, kind="stable")
    n_runs = W
    newid = np.full(N, -1, dtype=np.int64)
    d_of = np.full(N, -1, dtype=np.int64)
    p_of = np.full(N, -1, dtype=np.int64)
    run_of = np.full(N, -1, dtype=np.int64)
    L_runs = np.zeros(n_runs, dtype=np.int64)
    for r in range(n_runs):
        seg = order[r * RUN:(r + 1) * RUN]
        i = np.arange(len(seg))
        d = i % NDEV
        p = i // NDEV
        newid[seg] = d * P * W + p * W + r
        d_of[seg] = d
        p_of[seg] = p
        run_of[seg] = r
        L_runs[r] = int(deg_nl[seg].max()) + 1  # +1 self-loop slot
    assert (newid >= 0).all()
    off_runs = np.zeros(n_runs + 1, dtype=np.int64)
    np.cumsum(L_runs, out=off_runs[1:])
    F = int(off_runs[-1])

    # edges sorted by destination
    eorder = np.argsort(col, kind="stable")
    col_s = col[eorder]
    src_new = newid[row[eorder]]
    starts = np.zeros(N + 1, dtype=np.int64)
    np.cumsum(np.bincount(col_s, minlength=N), out=starts[1:])
    rank = np.arange(E, dtype=np.int64) - starts[col_s]

    d_e = d_of[col_s]
    p_e = p_of[col_s]
    slot_e = off_runs[run_of[col_s]] + 1 + rank
    line_tabs = np.zeros((NDEV, P, F), dtype=np.int16)
    offs_tabs = np.full((NDEV, P, F), 64.0, dtype=np.float32)
    line_tabs[d_e, p_e, slot_e] = (src_new // 64).astype(np.int16)
    offs_tabs[d_e, p_e, slot_e] = (src_new % 64).astype(np.float32)
    # self-loop slot at each dest's run start
    c = np.arange(N)
    self_slot = off_runs[run_of[c]]
    line_tabs[d_of[c], p_of[c], self_slot] = (newid[c] // 64).astype(np.int16)
    offs_tabs[d_of[c], p_of[c], self_slot] = (newid[c] % 64).astype(np.float32)

    # pieces: runs split to <= MAXLP slots
    pieces = []
    for r in range(n_runs):
        s = 0
        while s < L_runs[r]:
            lp = int(min(MAXLP, L_runs[r] - s))
            pieces.append((r, int(off_runs[r] + s), lp, s == 0))
            s += lp

    # wrapped idx tables [NDEV, P, 8F] int16
    idxs_tabs = np.zeros((NDEV, P, 8 * F), dtype=np.int16)
    for (r, base, lp, first) in pieces:
        ni = 128 * lp
        for d in range(NDEV):
            flat = line_tabs[d][:, base:base + lp].T.ravel()
            wrapped = flat.reshape(ni // 16, 16).T   # [16, ni//16]
            idxs_tabs[d][:, 8 * base:8 * (base + lp)] = np.tile(wrapped, (8, 1))

    deg_flat = np.zeros(TBL0, dtype=np.float32)
    deg_flat[newid] = deg.astype(np.float32)
    dinv2_flat = np.zeros(TBL0, dtype=np.float32)
    dinv2_flat[newid] = (1.0 / deg).astype(np.float32)
    u0_flat = np.zeros(TBL0, dtype=np.float32)
    u0_flat[newid] = (np.asarray(h).ravel() / np.sqrt(deg)).astype(np.float32)

    deg_sb = deg_flat.reshape(NDEV, P, W).transpose(1, 0, 2).reshape(P, WU).copy()
    dinv2l = dinv2_flat.reshape(NDEV, P, W)  # [d][p, r]

    meta = dict(F=F, pieces=pieces, L_runs=L_runs, newid=newid)
    arrays = dict(idxs_tabs=idxs_tabs, offs_tabs=offs_tabs, deg_sb=deg_sb,
                  dinv2l=dinv2l, u0_flat=u0_flat)
    return meta, arrays


def _build_nc(meta, layers=L_LAYERS):
    import concourse.bass as bass
    import concourse.mybir as mybir
    from concourse.tile import TileContext
    from concourse import library_config
    from concourse.library_overlay import lower_extended_insts

    F = meta["F"]
    pieces = meta["pieces"]

    nc = bass.Bass(num_devices=NDEV, num_swdge_queues=NQ)
    idxs = nc.dram_tensor("idxs", [P, 8 * F], mybir.dt.int16, kind="ExternalInput")
    offs = nc.dram_tensor("offs", [P, F], mybir.dt.float32, kind="ExternalInput")
    dinv2l_t = nc.dram_tensor("dinv2l", [P, W], mybir.dt.float32,
                              kind="ExternalInput")
    degsb_t = nc.dram_tensor("degsb", [P, WU], mybir.dt.float32,
                             kind="ExternalInput")
    iota_t = nc.dram_tensor("iota", [P, 64], mybir.dt.float32,
                            kind="ExternalInput")
    utab0 = nc.dram_tensor("utab0", [NLINES, 64], mybir.dt.float32,
                           kind="ExternalInput")
    c5_out = nc.dram_tensor("c5", [layers, 1], mybir.dt.float32,
                            kind="ExternalOutput")
    vchunks = [nc.dram_tensor(f"vchunk{l}", [P * W], mybir.dt.float32,
                              kind="Internal") for l in range(layers)]
    vgaths = [nc.dram_tensor(f"vgath{l}", [TBL0], mybir.dt.float32,
                             kind="Internal", addr_space="Shared")
              for l in range(layers)]

    with TileContext(nc) as tc:
        with tc.tile_pool(name="p", bufs=1) as pool, \
             tc.tile_pool(name="ps", bufs=1, space="PSUM") as psum:
            offs_sb = pool.tile([P, F], mybir.dt.float32, tag="offs")
            iota_sb = pool.tile([P, 64], mybir.dt.float32, tag="iota")
            dinv2_sb = pool.tile([P, W], mybir.dt.float32, tag="dinv2")
            deg_sb = pool.tile([P, WU], mybir.dt.float32, tag="deg")
            usb = pool.tile([P, WU], mybir.dt.float32, tag="usb")
            v = pool.tile([P, W], mybir.dt.float32, tag="v")
            un = pool.tile([P, W], mybir.dt.float32, tag="un")
            M = pool.tile([P, MAXLP * 64], mybir.dt.float32, tag="M")
            prod = pool.tile([P, MAXLP * 64], mybir.dt.float32, tag="prod")
            red = pool.tile([P, 1], mybir.dt.float32, tag="red")
            vtmp = pool.tile([P, 1], mybir.dt.float32, tag="vtmp")
            ones = pool.tile([P, 1], mybir.dt.float32, tag="ones")
            c5sb = pool.tile([1, 1], mybir.dt.float32, tag="c5sb")
            ps = psum.tile([1, 1], mybir.dt.float32, name="ps")
            Bq, idxq = [], []
            for q in range(NQ):
                bq = pool.tile([P, MAXLP * 64], mybir.dt.float32, tag=f"B{q}",
                               name=f"B{q}")
                Bq.append(bq)
                iq = pool.tile([P, 8 * MAXLP], mybir.dt.int16, tag=f"I{q}",
                               name=f"I{q}")
                idxq.append(iq)

            nc.sync.dma_start(offs_sb[:], offs[:])
            nc.sync.dma_start(iota_sb[:], iota_t[:])
            nc.sync.dma_start(dinv2_sb[:], dinv2l_t[:])
            nc.sync.dma_start(deg_sb[:], degsb_t[:])
            nc.vector.memset(ones[:], 1.0)
            nc.gpsimd.load_library(library_config.mlp)
            ni_regs = {}
            for (_r, _b, lp, _f) in pieces:
                if 128 * lp not in ni_regs:
                    ni_regs[128 * lp] = nc.gpsimd.to_reg(128 * lp)

            for l in range(layers):
                tab = utab0 if l == 0 else vgaths[l - 1]
                tab_ap = tab[:] if l == 0 else \
                    tab[:].rearrange("(a e) -> a e", e=64)
                with nc.named_scope(f"gather{l}"):
                    for j, (r, base, lp, first) in enumerate(pieces):
                        q = j % NQ
                        ni = 128 * lp
                        nc.sync.dma_start(idxq[q][:, :8 * lp],
                                          idxs[:, 8 * base:8 * (base + lp)])
                        nc.gpsimd.dma_gather(
                            out_ap=Bq[q][:, :lp * 64].rearrange(
                                "p (a e) -> p a e", e=64),
                            in_ap=tab_ap,
                            idxs_ap=idxq[q][:, :8 * lp],
                            num_idxs=ni, num_idxs_reg=ni_regs[ni], elem_size=64,
                            single_packet=False, queue_num=q)
                        nc.vector.tensor_tensor(
                            M[:, :lp * 64].rearrange("p (a e) -> p a e", e=64),
                            offs_sb[:, base:base + lp].rearrange(
                                "p (a e) -> p a e", e=1).to_broadcast(
                                [P, lp, 64]),
                            iota_sb[:].rearrange("p (a e) -> p a e", a=1)
                            .to_broadcast([P, lp, 64]),
                            mybir.AluOpType.is_equal)
                        nc.vector.tensor_tensor(
                            prod[:, :lp * 64], Bq[q][:, :lp * 64],
                            M[:, :lp * 64], mybir.AluOpType.mult)
                        tgt = v[:, r:r + 1] if first else vtmp[:]
                        nc.vector.tensor_reduce(
                            tgt, prod[:, :lp * 64].rearrange(
                                "p (a e) -> p a e", a=1),
                            op=mybir.AluOpType.add,
                            axis=mybir.AxisListType.X)
                        if not first:
                            nc.vector.tensor_tensor(
                                v[:, r:r + 1], v[:, r:r + 1], vtmp[:],
                                mybir.AluOpType.add)
                with nc.named_scope(f"update{l}"):
                    nc.vector.tensor_tensor(un[:], v[:], dinv2_sb[:],
                                            mybir.AluOpType.mult)
                    nc.sync.dma_start(
                        vchunks[l][:].rearrange("(p w) -> p w", p=P), un[:])
                    nc.gpsimd.collective_compute(
                        "AllGather", mybir.AluOpType.bypass,
                        replica_groups=[list(range(NDEV))],
                        ins=[vchunks[l][:]], outs=[vgaths[l][:]])
                    nc.sync.dma_start(
                        usb[:].rearrange("p (d w) -> p d w", d=NDEV),
                        vgaths[l][:].rearrange("(d p w) -> p d w", d=NDEV, p=P))
                    nc.vector.tensor_tensor(usb[:], usb[:], usb[:],
                                            mybir.AluOpType.mult)
                    nc.vector.tensor_tensor(usb[:], usb[:], deg_sb[:],
                                            mybir.AluOpType.mult)
                    nc.vector.tensor_reduce(red[:], usb[:],
                                            op=mybir.AluOpType.add,
                                            axis=mybir.AxisListType.X)
                    nc.tensor.matmul(ps[:], red[:], ones[:], start=True,
                                     stop=True)
                    nc.vector.tensor_copy(c5sb[:], ps[:])
                    nc.sync.dma_start(c5_out[l:l + 1, :], c5sb[:])
    lower_extended_insts(nc)
    legalize_waits(nc)
    return nc


def kernel(h, edge_index):
    from concourse import bass_utils

    h = np.asarray(h, dtype=np.float32)
    edge_index = np.asarray(edge_index)
    meta, arrays = _host_prep(h, edge_index)
    if "k" not in _cache:
        _cache["k"] = _build_nc(meta)
    nc = _cache["k"]

    iota = np.tile(np.arange(64, dtype=np.float32), (P, 1))
    in_maps = []
    for d in range(NDEV):
        in_maps.append(dict(
            idxs=arrays["idxs_tabs"][d],
            offs=arrays["offs_tabs"][d],
            dinv2l=arrays["dinv2l"][d],
            degsb=arrays["deg_sb"],
            iota=iota,
            utab0=arrays["u0_flat"].reshape(NLINES, 64),
        ))
    res = bass_utils.run_bass_kernel_spmd(
        nc, in_maps, core_ids=list(range(NDEV)), trace=False)
    return res.results[0]["c5"][:, 0].copy()

